# revision 18
# baseline (speedup 1.0000x reference)
"""Trainium2 Bass kernel for ContextQueryAttention (trilinear similarity +
row/col softmax attention).

Full-input contract: kernel(**inputs) takes the complete arrays
  q  [16, 128, 512]   f32
  c  [16, 128, 4096]  f32
  w1 [1, 128] w2 [1, 128] w3 [1, 128] f32
and returns out [16, 512, 4096] f32 = concat([c, a, c*a, c*b], axis=1).

Sharding: data-parallel over batch B=16 across 8 NeuronCores (2 batches per
core), no collectives.

Structure: streaming n-windows of 1024 across both batches (8 global
windows), software-pipelined so PE / ACT / DVE / GPSIMD / DMA overlap:

  per window w (8 n-chunks of 128):
    pass2:  s' = A^T q per chunk -> ACT exp -> expS [n,m] (unscaled; the
            e^{out2[n]} factor cancels in the row softmax)
    rowsum: one DVE reduce per window (off the PE critical path), then
            rowinv = 1/(rowsum*e2) -> PE transpose -> [1,1024] row ->
            gpsimd partition_broadcast
    cE:     PE chunk transposes of c, drained with a per-chunk
            tensor_scalar that folds e2[n] in (cE = c^T * e2)
    tmp:    accumulates cE^T @ expS over all windows (PSUM resident)
    pass1:  s^T = Bq^T c + out1[m] bias -> ACT exp (true exp, FD1024) with
            accum_out -> colsum partials
    a-wave: qT^T @ expST at end of the window (FD1024); its rowinv
            normalization + ca = a*c (gpsimd) land at the head of the next
            window so they never head-of-line block anything
  batch b's b-waves (tmpT^T @ expST, cb = c*b) ride along in batch b+1's
  windows / the tail.

All matmul operands bf16; exp outputs written bf16 by ACT directly; out
block 0 (the c passthrough) is DRAM->DRAM DMA, untouched by compute.
"""

import sys

import numpy as np

try:
    import concourse.bass as bass  # noqa: F401
except Exception:  # pragma: no cover
    sys.path.insert(0, "/opt/trn_rl_repo")
    import concourse.bass as bass  # noqa: F401

import concourse.bacc as bacc
import concourse.mybir as mybir
import concourse.tile as tile
from concourse.masks import make_identity

F32 = mybir.dt.float32
BF16 = mybir.dt.bfloat16

# Problem geometry (hardcoded per contract)
B = 16          # total batches
NCORES = 8
CB = B // NCORES  # batches per core = 2
D = 128         # model dim == partition count
M = 512         # query length
N = 4096        # context length
P = 128
MCH = M // P    # 4 m-chunks of 128
WIN = 1024      # n-window width
NWIN = N // WIN  # 4 windows per batch
CHW = WIN // P   # 8 n-chunks per window
T = CB * NWIN    # 8 global windows


def build_body(tc, q_ap, c_ap, w1_ap, w2_ap, w3_ap, out_ap):
    """Emit the per-core program. q_ap [CB,128,512], c_ap [CB,128,4096],
    w*_ap [1,128], out_ap [CB,512,4096]."""
    from contextlib import ExitStack

    nc = tc.nc
    mult = mybir.AluOpType.mult
    add = mybir.AluOpType.add
    Exp = mybir.ActivationFunctionType.Exp
    AxX = mybir.AxisListType.X

    with ExitStack() as ctx:
        consts = ctx.enter_context(tc.tile_pool(name="consts", bufs=1))
        per = ctx.enter_context(tc.tile_pool(name="per", bufs=2))
        win = ctx.enter_context(tc.tile_pool(name="win", bufs=2))
        outp = ctx.enter_context(tc.tile_pool(name="outp", bufs=2))
        # PSUM: "s" ring (pass1/pass2, 2-bank tiles x2), "w" ring (waves /
        # transposes / o2, 1-bank tiles x2), tmp accumulator (1 bank)
        pp_s = ctx.enter_context(tc.tile_pool(name="pp_s", bufs=2, space="PSUM"))
        pp_w = ctx.enter_context(tc.tile_pool(name="pp_w", bufs=2, space="PSUM"))
        pp_acc = ctx.enter_context(tc.tile_pool(name="pp_acc", bufs=1, space="PSUM"))

        identity = consts.tile([P, P], F32)
        make_identity(nc, identity)
        identity_b = consts.tile([P, P], BF16)
        nc.vector.tensor_copy(identity_b, identity)
        w1c = consts.tile([P, 1], F32)
        w2c = consts.tile([P, 1], F32)
        w3c = consts.tile([P, 1], F32)
        nc.sync.dma_start(out=w1c, in_=w1_ap.rearrange("o d -> d o"))
        nc.sync.dma_start(out=w2c, in_=w2_ap.rearrange("o d -> d o"))
        nc.sync.dma_start(out=w3c, in_=w3_ap.rearrange("o d -> d o"))
        w1b = consts.tile([P, 1], BF16)
        w2b = consts.tile([P, 1], BF16)
        nc.vector.tensor_copy(w1b, w1c)
        nc.vector.tensor_copy(w2b, w2c)

        state = [dict() for _ in range(CB)]
        pend_a = []
        pend_b = []
        stash = {}   # t -> dict(A, cb, cE, expS, riC)

        def prep_load(b):
            st = state[b]
            st["c_t"] = per.tile([P, N], F32, tag="ct", name=f"ct{b}")
            st["q_b"] = per.tile([P, M], BF16, tag="qb", name=f"qb{b}")
            nc.gpsimd.dma_start(out=st["q_b"], in_=q_ap[b])

        def load_cwin(t):
            b, w = divmod(t, NWIN)
            lo = w * WIN
            nc.sync.dma_start(
                out=state[b]["c_t"][:, lo : lo + WIN],
                in_=c_ap[b][:, lo : lo + WIN],
            )

        def hoist_prep(t):
            """A and bf16-c for window t, emitted one window early so the
            next window's pass2 never waits on this window's DVE queue."""
            b, w = divmod(t, NWIN)
            st = state[b]
            lo = w * WIN
            A_w = win.tile([P, WIN], BF16, tag="A")
            nc.vector.tensor_scalar(
                A_w, st["c_t"][:, lo : lo + WIN], w3c, w1c, mult, add
            )
            c_b_w = win.tile([P, WIN], BF16, tag="cbw")
            nc.vector.tensor_copy(c_b_w, st["c_t"][:, lo : lo + WIN])
            stash.setdefault(t, {})["A"] = A_w
            stash[t]["cb"] = c_b_w

        def prep_compute(b):
            st = state[b]
            q_b = st["q_b"]
            qT = per.tile([P, M], BF16, tag="qT", name=f"qT{b}")
            for i in range(MCH):
                ps_q = pp_w.tile([P, P], BF16, tag="w")
                nc.tensor.transpose(ps_q, q_b[:, i * P : (i + 1) * P], identity_b)
                nc.vector.tensor_copy(qT[:, i * P : (i + 1) * P], ps_q)
            st["qT"] = qT
            Bq = per.tile([P, M], BF16, tag="Bq", name=f"Bq{b}")
            nc.vector.tensor_scalar(Bq, q_b, w3c, w2c, mult, add)
            st["Bq"] = Bq
            ps_o1 = pp_w.tile([P, MCH], F32, tag="w")
            for i in range(MCH):
                nc.tensor.matmul(
                    ps_o1[:, i : i + 1],
                    lhsT=q_b[:, i * P : (i + 1) * P],
                    rhs=w1b,
                    start=True,
                    stop=True,
                )
            o1col = per.tile([P, MCH], F32, tag="o1", name=f"o1{b}")
            nc.vector.tensor_copy(o1col, ps_o1)
            st["o1col"] = o1col
            st["expST"] = per.tile([P, MCH, N], BF16, tag="expST", name=f"eST{b}")
            st["rowinvb"] = per.tile([P, N], BF16, tag="rowinvb", name=f"riv{b}")
            st["tmp_ps"] = pp_acc.tile([P, M], F32, tag="tmp", name=f"tmp{b}")
            st["colsumU"] = per.tile([P, MCH, NWIN], F32, tag="csU", name=f"csU{b}")

        def rowinv_tail(t):
            """Transpose + broadcast of window t's rowinv columns; emitted at
            the head of window t+1 (recip(t) finished by then)."""
            b, w = divmod(t, NWIN)
            st = state[b]
            lo = w * WIN
            riC = stash[t]["riC"]
            ps_rT = pp_w.tile([CHW, P], F32, tag="w")
            nc.tensor.transpose(ps_rT, riC, identity)
            rowT_w = win.tile([CHW, P], BF16, tag="rT")
            nc.vector.tensor_copy(rowT_w, ps_rT)
            rowrow_w = win.tile([1, WIN], BF16, tag="rr")
            nc.sync.dma_start(
                out=rowrow_w.rearrange("p (a b) -> p a b", a=CHW), in_=rowT_w
            )
            nc.gpsimd.partition_broadcast(st["rowinvb"][:, lo : lo + WIN], rowrow_w)

        def tmp_mms(t):
            b, w = divmod(t, NWIN)
            st = state[b]
            cE_w = stash[t]["cE"]
            expS_w = stash[t]["expS"]
            for j in range(CHW):
                nc.tensor.matmul(
                    st["tmp_ps"],
                    lhsT=cE_w[:, j, :],
                    rhs=expS_w[:, j, :],
                    start=(w == 0 and j == 0),
                    stop=(w == NWIN - 1 and j == CHW - 1),
                )

        def awave_mm(t):
            b, w = divmod(t, NWIN)
            st = state[b]
            lo = w * WIN
            for h in range(2):
                l2 = lo + h * M
                ps_a = pp_w.tile([P, M], F32, tag="w")
                for i in range(MCH):
                    nc.tensor.matmul(
                        ps_a,
                        lhsT=st["qT"][:, i * P : (i + 1) * P],
                        rhs=st["expST"][:, i, l2 : l2 + M],
                        start=(i == 0),
                        stop=(i == MCH - 1),
                    )
                pend_a.append((ps_a, b, l2))

        def awave_finish():
            while pend_a:
                ps_a, b, l2 = pend_a.pop(0)
                st = state[b]
                a_t = outp.tile([P, M], F32, tag="a")
                nc.vector.tensor_tensor(
                    a_t, ps_a, st["rowinvb"][:, l2 : l2 + M], mult
                )
                nc.sync.dma_start(out=out_ap[b, P : 2 * P, l2 : l2 + M], in_=a_t)
                ca_t = outp.tile([P, M], F32, tag="ca")
                nc.gpsimd.tensor_tensor(ca_t, a_t, st["c_t"][:, l2 : l2 + M], mult)
                nc.sync.dma_start(
                    out=out_ap[b, 2 * P : 3 * P, l2 : l2 + M], in_=ca_t
                )

        def emit_bprep(b):
            st = state[b]
            tmpU = per.tile([P, M], BF16, tag="tmpU", name=f"tmpU{b}")
            nc.vector.tensor_copy(tmpU, st["tmp_ps"])
            colsum = per.tile([P, MCH], F32, tag="cs", name=f"cs{b}")
            nc.vector.reduce_sum(colsum, st["colsumU"], axis=AxX)
            colinv = per.tile([P, MCH], F32, tag="colinv", name=f"cinv{b}")
            nc.vector.reciprocal(colinv, colsum)
            tmpT = per.tile([P, M], BF16, tag="tmpT", name=f"tmpT{b}")
            for i in range(MCH):
                ps_tt = pp_w.tile([P, P], BF16, tag="w")
                nc.tensor.transpose(ps_tt, tmpU[:, i * P : (i + 1) * P], identity_b)
                nc.vector.tensor_scalar(
                    tmpT[:, i * P : (i + 1) * P],
                    ps_tt,
                    colinv[:, i : i + 1],
                    None,
                    mult,
                )
            st["tmpT"] = tmpT

        def bwave_mm(b, w):
            st = state[b]
            lo = w * WIN
            for h in range(2):
                l2 = lo + h * M
                ps_b = pp_w.tile([P, M], F32, tag="w")
                for i in range(MCH):
                    nc.tensor.matmul(
                        ps_b,
                        lhsT=st["tmpT"][:, i * P : (i + 1) * P],
                        rhs=st["expST"][:, i, l2 : l2 + M],
                        start=(i == 0),
                        stop=(i == MCH - 1),
                    )
                pend_b.append((ps_b, b, l2))

        def bwave_finish():
            while pend_b:
                ps_b, b, l2 = pend_b.pop(0)
                st = state[b]
                b1_t = outp.tile([P, M], F32, tag="b1")
                nc.vector.tensor_tensor(
                    b1_t, ps_b, st["rowinvb"][:, l2 : l2 + M], mult
                )
                cb_t = outp.tile([P, M], F32, tag="cb")
                nc.gpsimd.tensor_tensor(cb_t, b1_t, st["c_t"][:, l2 : l2 + M], mult)
                nc.sync.dma_start(
                    out=out_ap[b, 3 * P : 4 * P, l2 : l2 + M], in_=cb_t
                )

        def emit_window(t):
            b, w = divmod(t, NWIN)
            st = state[b]
            lo = w * WIN

            # ---- prologue: previous window's rowinv broadcast chain ----
            if t > 0:
                rowinv_tail(t - 1)
            if t + 2 < T:
                if (t + 2) % NWIN == 0:
                    prep_load((t + 2) // NWIN)
                load_cwin(t + 2)
            nc.sync.dma_start(
                out=out_ap[b, 0:P, lo : lo + WIN], in_=c_ap[b][:, lo : lo + WIN]
            )
            if w == 0:
                prep_compute(b)

            # ---- pass 2 (A was hoisted a window ago) ----
            A_w = stash[t]["A"]
            c_b_w = stash[t]["cb"]
            expS_w = win.tile([P, CHW, M], BF16, tag="expS")
            for jj in range(CHW // 2):
                ps2 = pp_s.tile([P, 2 * M], F32, tag="s")
                for h in range(2):
                    j = 2 * jj + h
                    nc.tensor.matmul(
                        ps2[:, h * M : (h + 1) * M],
                        lhsT=A_w[:, j * P : (j + 1) * P],
                        rhs=st["q_b"],
                        start=True,
                        stop=True,
                    )
                nc.scalar.activation(
                    expS_w[:, 2 * jj : 2 * jj + 2, :], ps2, Exp, bias=0.0, scale=1.0
                )

            # ---- out2 -> e2 ----
            ps_o2 = pp_w.tile([P, CHW], F32, tag="w")
            for j in range(CHW):
                nc.tensor.matmul(
                    ps_o2[:, j : j + 1],
                    lhsT=c_b_w[:, j * P : (j + 1) * P],
                    rhs=w2b,
                    start=True,
                    stop=True,
                )
            e2col_w = win.tile([P, CHW], F32, tag="e2")
            nc.scalar.activation(e2col_w, ps_o2, Exp, bias=0.0, scale=1.0)

            # ---- hoist next window's A/cast (ahead of this window's DVE
            # reduce/norm backlog) ----
            if t + 1 < T:
                hoist_prep(t + 1)

            # ---- cE = c^T * e2 ----
            cE_w = win.tile([P, CHW, P], BF16, tag="cE")
            for x in range(CHW // 2):
                ps_ct = pp_w.tile([P, 2, P], BF16, tag="w")
                for k in range(2):
                    j = 2 * x + k
                    nc.tensor.transpose(
                        ps_ct[:, k, :], c_b_w[:, j * P : (j + 1) * P], identity_b
                    )
                for k in range(2):
                    j = 2 * x + k
                    nc.vector.tensor_scalar(
                        cE_w[:, j, :],
                        ps_ct[:, k, :],
                        e2col_w[:, j : j + 1],
                        None,
                        mult,
                    )
            stash[t]["cE"] = cE_w
            stash[t]["expS"] = expS_w

            # ---- previous window's tmp accumulation ----
            if t > 0:
                tmp_mms(t - 1)
            if w == 0 and b == 1:
                emit_bprep(0)

            # ---- pass 1 ----
            for i in range(MCH):
                ps1 = pp_s.tile([P, WIN], F32, tag="s")
                for h in range(2):
                    nc.tensor.matmul(
                        ps1[:, h * M : (h + 1) * M],
                        lhsT=st["Bq"][:, i * P : (i + 1) * P],
                        rhs=c_b_w[:, h * M : (h + 1) * M],
                        start=True,
                        stop=True,
                    )
                nc.scalar.activation(
                    st["expST"][:, i, lo : lo + WIN],
                    ps1,
                    Exp,
                    bias=st["o1col"][:, i : i + 1],
                    scale=1.0,
                    accum_out=st["colsumU"][:, i, w : w + 1],
                )

            # ---- waves ----
            if b == 1:
                bwave_mm(0, w)
                bwave_finish()
            if t > 0:
                awave_mm(t - 1)
                awave_finish()

            # ---- rowsum -> rowinv columns (tail of the DVE queue; its
            # broadcast happens in the next window's prologue) ----
            rowsumC_w = win.tile([P, CHW], F32, tag="rs")
            nc.vector.reduce_sum(rowsumC_w, expS_w, axis=AxX)
            rowprod_w = win.tile([P, CHW], F32, tag="rp")
            nc.vector.tensor_tensor(rowprod_w, rowsumC_w, e2col_w, mult)
            rowinvC_w = win.tile([P, CHW], F32, tag="ri")
            nc.vector.reciprocal(rowinvC_w, rowprod_w)
            stash[t]["riC"] = rowinvC_w

        # ---- fill ----
        prep_load(0)
        load_cwin(0)
        load_cwin(1)
        hoist_prep(0)
        for t in range(T):
            emit_window(t)
        # ---- drain ----
        rowinv_tail(T - 1)
        tmp_mms(T - 1)
        awave_mm(T - 1)
        awave_finish()
        emit_bprep(1)
        for w in range(NWIN):
            bwave_mm(1, w)
            bwave_finish()


_PROGRAM = None


def _build_program(loops=None):
    """Build the per-core Bass program. loops=None -> straight-line (grading
    path); loops=R -> wrap the body in a Tile For_i repetition loop (used
    only for steady-state benchmarking)."""
    nc = bacc.Bacc("TRN2", target_bir_lowering=False, debug=False, num_devices=NCORES)
    q_d = nc.dram_tensor("q", [CB, D, M], F32, kind="ExternalInput")
    c_d = nc.dram_tensor("c", [CB, D, N], F32, kind="ExternalInput")
    w1_d = nc.dram_tensor("w1", [1, D], F32, kind="ExternalInput")
    w2_d = nc.dram_tensor("w2", [1, D], F32, kind="ExternalInput")
    w3_d = nc.dram_tensor("w3", [1, D], F32, kind="ExternalInput")
    out_d = nc.dram_tensor("out", [CB, 4 * D, N], F32, kind="ExternalOutput")
    with tile.TileContext(nc) as tc:
        if loops is None:
            build_body(
                tc, q_d.ap(), c_d.ap(), w1_d.ap(), w2_d.ap(), w3_d.ap(), out_d.ap()
            )
        else:
            with tc.For_i(0, loops, 1):
                build_body(
                    tc,
                    q_d.ap(),
                    c_d.ap(),
                    w1_d.ap(),
                    w2_d.ap(),
                    w3_d.ap(),
                    out_d.ap(),
                )
    nc.compile()
    return nc


def _get_program():
    global _PROGRAM
    if _PROGRAM is None:
        _PROGRAM = _build_program()
    return _PROGRAM


def kernel(q, c, w1, w2, w3, _collect_results=None):
    q = np.ascontiguousarray(q, dtype=np.float32)
    c = np.ascontiguousarray(c, dtype=np.float32)
    w1 = np.ascontiguousarray(w1, dtype=np.float32)
    w2 = np.ascontiguousarray(w2, dtype=np.float32)
    w3 = np.ascontiguousarray(w3, dtype=np.float32)

    nc = _get_program()
    in_maps = [
        {
            "q": q[CB * i : CB * (i + 1)],
            "c": c[CB * i : CB * (i + 1)],
            "w1": w1,
            "w2": w2,
            "w3": w3,
        }
        for i in range(NCORES)
    ]
    from concourse import bass_utils

    res = bass_utils.run_bass_kernel_spmd(nc, in_maps, core_ids=list(range(NCORES)))
    if _collect_results is not None:
        _collect_results.append(res)
    return np.concatenate([r["out"] for r in res.results], axis=0)


# revision 19
# speedup vs baseline: 1.0153x; 1.0153x over previous
"""Trainium2 Bass kernel for ContextQueryAttention (trilinear similarity +
row/col softmax attention).

Full-input contract: kernel(**inputs) takes the complete arrays
  q  [16, 128, 512]   f32
  c  [16, 128, 4096]  f32
  w1 [1, 128] w2 [1, 128] w3 [1, 128] f32
and returns out [16, 512, 4096] f32 = concat([c, a, c*a, c*b], axis=1).

Sharding: data-parallel over batch B=16 across 8 NeuronCores (2 batches per
core), no collectives.

Structure: streaming n-windows of 1024 across both batches (8 global
windows), software-pipelined so PE / ACT / DVE / GPSIMD / DMA overlap:

  per window w (8 n-chunks of 128):
    pass2:  s' = A^T q per chunk -> ACT exp -> expS [n,m] (unscaled; the
            e^{out2[n]} factor cancels in the row softmax)
    rowsum: one DVE reduce per window (off the PE critical path), then
            rowinv = 1/(rowsum*e2) -> PE transpose -> [1,1024] row ->
            gpsimd partition_broadcast
    cE:     PE chunk transposes of c, drained with a per-chunk
            tensor_scalar that folds e2[n] in (cE = c^T * e2)
    tmp:    accumulates cE^T @ expS over all windows (PSUM resident)
    pass1:  s^T = Bq^T c + out1[m] bias -> ACT exp (true exp, FD1024) with
            accum_out -> colsum partials
    a-wave: qT^T @ expST at end of the window (FD1024); its rowinv
            normalization + ca = a*c (gpsimd) land at the head of the next
            window so they never head-of-line block anything
  batch b's b-waves (tmpT^T @ expST, cb = c*b) ride along in batch b+1's
  windows / the tail.

All matmul operands bf16; exp outputs written bf16 by ACT directly; out
block 0 (the c passthrough) is DRAM->DRAM DMA, untouched by compute.
"""

import sys

import numpy as np

try:
    import concourse.bass as bass  # noqa: F401
except Exception:  # pragma: no cover
    sys.path.insert(0, "/opt/trn_rl_repo")
    import concourse.bass as bass  # noqa: F401

import concourse.bacc as bacc
import concourse.mybir as mybir
import concourse.tile as tile
from concourse.masks import make_identity

F32 = mybir.dt.float32
BF16 = mybir.dt.bfloat16

# Problem geometry (hardcoded per contract)
B = 16          # total batches
NCORES = 8
CB = B // NCORES  # batches per core = 2
D = 128         # model dim == partition count
M = 512         # query length
N = 4096        # context length
P = 128
MCH = M // P    # 4 m-chunks of 128
WIN = 1024      # n-window width
NWIN = N // WIN  # 4 windows per batch
CHW = WIN // P   # 8 n-chunks per window
T = CB * NWIN    # 8 global windows


def build_body(tc, q_ap, c_ap, w1_ap, w2_ap, w3_ap, out_ap):
    """Emit the per-core program. q_ap [CB,128,512], c_ap [CB,128,4096],
    w*_ap [1,128], out_ap [CB,512,4096]."""
    from contextlib import ExitStack

    nc = tc.nc
    mult = mybir.AluOpType.mult
    add = mybir.AluOpType.add
    Exp = mybir.ActivationFunctionType.Exp
    AxX = mybir.AxisListType.X

    with ExitStack() as ctx:
        consts = ctx.enter_context(tc.tile_pool(name="consts", bufs=1))
        per = ctx.enter_context(tc.tile_pool(name="per", bufs=2))
        win = ctx.enter_context(tc.tile_pool(name="win", bufs=2))
        outp = ctx.enter_context(tc.tile_pool(name="outp", bufs=2))
        # PSUM: "s" ring (pass1/pass2, 2-bank tiles x2), "w" ring (waves /
        # transposes / o2, 1-bank tiles x2), tmp accumulator (1 bank)
        pp_s = ctx.enter_context(tc.tile_pool(name="pp_s", bufs=2, space="PSUM"))
        pp_w = ctx.enter_context(tc.tile_pool(name="pp_w", bufs=3, space="PSUM"))
        pp_acc = ctx.enter_context(tc.tile_pool(name="pp_acc", bufs=1, space="PSUM"))

        identity = consts.tile([P, P], F32)
        make_identity(nc, identity)
        identity_b = consts.tile([P, P], BF16)
        nc.vector.tensor_copy(identity_b, identity)
        w1c = consts.tile([P, 1], F32)
        w2c = consts.tile([P, 1], F32)
        w3c = consts.tile([P, 1], F32)
        nc.sync.dma_start(out=w1c, in_=w1_ap.rearrange("o d -> d o"))
        nc.sync.dma_start(out=w2c, in_=w2_ap.rearrange("o d -> d o"))
        nc.sync.dma_start(out=w3c, in_=w3_ap.rearrange("o d -> d o"))
        w1b = consts.tile([P, 1], BF16)
        w2b = consts.tile([P, 1], BF16)
        nc.vector.tensor_copy(w1b, w1c)
        nc.vector.tensor_copy(w2b, w2c)

        state = [dict() for _ in range(CB)]
        pend_a = []
        pend_b = []
        stash = {}   # t -> dict(A, cb, cE, expS, riC)

        def prep_load(b):
            st = state[b]
            st["c_t"] = per.tile([P, N], F32, tag="ct", name=f"ct{b}")
            st["q_b"] = per.tile([P, M], BF16, tag="qb", name=f"qb{b}")
            nc.gpsimd.dma_start(out=st["q_b"], in_=q_ap[b])

        def load_cwin(t):
            b, w = divmod(t, NWIN)
            lo = w * WIN
            nc.sync.dma_start(
                out=state[b]["c_t"][:, lo : lo + WIN],
                in_=c_ap[b][:, lo : lo + WIN],
            )

        def hoist_prep(t):
            """A and bf16-c for window t, emitted one window early so the
            next window's pass2 never waits on this window's DVE queue."""
            b, w = divmod(t, NWIN)
            st = state[b]
            lo = w * WIN
            A_w = win.tile([P, WIN], BF16, tag="A")
            nc.vector.tensor_scalar(
                A_w, st["c_t"][:, lo : lo + WIN], w3c, w1c, mult, add
            )
            c_b_w = win.tile([P, WIN], BF16, tag="cbw")
            nc.vector.tensor_copy(c_b_w, st["c_t"][:, lo : lo + WIN])
            stash.setdefault(t, {})["A"] = A_w
            stash[t]["cb"] = c_b_w

        def prep_compute(b):
            st = state[b]
            q_b = st["q_b"]
            qT = per.tile([P, M], BF16, tag="qT", name=f"qT{b}")
            for i in range(MCH):
                ps_q = pp_w.tile([P, P], BF16, tag="w")
                nc.tensor.transpose(ps_q, q_b[:, i * P : (i + 1) * P], identity_b)
                nc.vector.tensor_copy(qT[:, i * P : (i + 1) * P], ps_q)
            st["qT"] = qT
            Bq = per.tile([P, M], BF16, tag="Bq", name=f"Bq{b}")
            nc.vector.tensor_scalar(Bq, q_b, w3c, w2c, mult, add)
            st["Bq"] = Bq
            ps_o1 = pp_w.tile([P, MCH], F32, tag="w")
            for i in range(MCH):
                nc.tensor.matmul(
                    ps_o1[:, i : i + 1],
                    lhsT=q_b[:, i * P : (i + 1) * P],
                    rhs=w1b,
                    start=True,
                    stop=True,
                )
            o1col = per.tile([P, MCH], F32, tag="o1", name=f"o1{b}")
            nc.vector.tensor_copy(o1col, ps_o1)
            st["o1col"] = o1col
            st["expST"] = per.tile([P, MCH, N], BF16, tag="expST", name=f"eST{b}")
            st["rowinvb"] = per.tile([P, N], BF16, tag="rowinvb", name=f"riv{b}")
            st["tmp_ps"] = pp_acc.tile([P, M], F32, tag="tmp", name=f"tmp{b}")
            st["colsumU"] = per.tile([P, MCH, NWIN], F32, tag="csU", name=f"csU{b}")

        def rowinv_tail(t):
            """Transpose + broadcast of window t's rowinv columns; emitted at
            the head of window t+1 (recip(t) finished by then)."""
            b, w = divmod(t, NWIN)
            st = state[b]
            lo = w * WIN
            riC = stash[t]["riC"]
            ps_rT = pp_w.tile([CHW, P], F32, tag="w")
            nc.tensor.transpose(ps_rT, riC, identity)
            rowT_w = win.tile([CHW, P], BF16, tag="rT")
            nc.vector.tensor_copy(rowT_w, ps_rT)
            rowrow_w = win.tile([1, WIN], BF16, tag="rr")
            nc.sync.dma_start(
                out=rowrow_w.rearrange("p (a b) -> p a b", a=CHW), in_=rowT_w
            )
            nc.gpsimd.partition_broadcast(st["rowinvb"][:, lo : lo + WIN], rowrow_w)

        def tmp_mms(t):
            b, w = divmod(t, NWIN)
            st = state[b]
            cE_w = stash[t]["cE"]
            expS_w = stash[t]["expS"]
            for j in range(CHW):
                nc.tensor.matmul(
                    st["tmp_ps"],
                    lhsT=cE_w[:, j, :],
                    rhs=expS_w[:, j, :],
                    start=(w == 0 and j == 0),
                    stop=(w == NWIN - 1 and j == CHW - 1),
                )

        def awave_mm(t):
            b, w = divmod(t, NWIN)
            st = state[b]
            lo = w * WIN
            for h in range(2):
                l2 = lo + h * M
                ps_a = pp_w.tile([P, M], F32, tag="w")
                for i in range(MCH):
                    nc.tensor.matmul(
                        ps_a,
                        lhsT=st["qT"][:, i * P : (i + 1) * P],
                        rhs=st["expST"][:, i, l2 : l2 + M],
                        start=(i == 0),
                        stop=(i == MCH - 1),
                    )
                pend_a.append((ps_a, b, l2))

        def awave_finish():
            while pend_a:
                ps_a, b, l2 = pend_a.pop(0)
                st = state[b]
                a_t = outp.tile([P, M], F32, tag="a")
                nc.vector.tensor_tensor(
                    a_t, ps_a, st["rowinvb"][:, l2 : l2 + M], mult
                )
                nc.sync.dma_start(out=out_ap[b, P : 2 * P, l2 : l2 + M], in_=a_t)
                ca_t = outp.tile([P, M], F32, tag="ca")
                nc.gpsimd.tensor_tensor(ca_t, a_t, st["c_t"][:, l2 : l2 + M], mult)
                nc.sync.dma_start(
                    out=out_ap[b, 2 * P : 3 * P, l2 : l2 + M], in_=ca_t
                )

        def emit_bprep(b):
            st = state[b]
            tmpU = per.tile([P, M], BF16, tag="tmpU", name=f"tmpU{b}")
            nc.vector.tensor_copy(tmpU, st["tmp_ps"])
            colsum = per.tile([P, MCH], F32, tag="cs", name=f"cs{b}")
            nc.vector.reduce_sum(colsum, st["colsumU"], axis=AxX)
            colinv = per.tile([P, MCH], F32, tag="colinv", name=f"cinv{b}")
            nc.vector.reciprocal(colinv, colsum)
            tmpT = per.tile([P, M], BF16, tag="tmpT", name=f"tmpT{b}")
            for i in range(MCH):
                ps_tt = pp_w.tile([P, P], BF16, tag="w")
                nc.tensor.transpose(ps_tt, tmpU[:, i * P : (i + 1) * P], identity_b)
                nc.vector.tensor_scalar(
                    tmpT[:, i * P : (i + 1) * P],
                    ps_tt,
                    colinv[:, i : i + 1],
                    None,
                    mult,
                )
            st["tmpT"] = tmpT

        def bwave_mm(b, w):
            st = state[b]
            lo = w * WIN
            for h in range(2):
                l2 = lo + h * M
                ps_b = pp_w.tile([P, M], F32, tag="w")
                for i in range(MCH):
                    nc.tensor.matmul(
                        ps_b,
                        lhsT=st["tmpT"][:, i * P : (i + 1) * P],
                        rhs=st["expST"][:, i, l2 : l2 + M],
                        start=(i == 0),
                        stop=(i == MCH - 1),
                    )
                pend_b.append((ps_b, b, l2))

        def bwave_finish():
            while pend_b:
                ps_b, b, l2 = pend_b.pop(0)
                st = state[b]
                b1_t = outp.tile([P, M], F32, tag="b1")
                nc.vector.tensor_tensor(
                    b1_t, ps_b, st["rowinvb"][:, l2 : l2 + M], mult
                )
                cb_t = outp.tile([P, M], F32, tag="cb")
                nc.gpsimd.tensor_tensor(cb_t, b1_t, st["c_t"][:, l2 : l2 + M], mult)
                nc.sync.dma_start(
                    out=out_ap[b, 3 * P : 4 * P, l2 : l2 + M], in_=cb_t
                )

        def emit_window(t):
            b, w = divmod(t, NWIN)
            st = state[b]
            lo = w * WIN

            # ---- prologue ----
            if t + 2 < T:
                if (t + 2) % NWIN == 0:
                    prep_load((t + 2) // NWIN)
                load_cwin(t + 2)
            nc.sync.dma_start(
                out=out_ap[b, 0:P, lo : lo + WIN], in_=c_ap[b][:, lo : lo + WIN]
            )
            if w == 0:
                prep_compute(b)

            # ---- pass 2 (A was hoisted a window ago) ----
            A_w = stash[t]["A"]
            c_b_w = stash[t]["cb"]
            expS_w = win.tile([P, CHW, M], BF16, tag="expS")
            for jj in range(CHW // 2):
                ps2 = pp_s.tile([P, 2 * M], F32, tag="s")
                for h in range(2):
                    j = 2 * jj + h
                    nc.tensor.matmul(
                        ps2[:, h * M : (h + 1) * M],
                        lhsT=A_w[:, j * P : (j + 1) * P],
                        rhs=st["q_b"],
                        start=True,
                        stop=True,
                    )
                nc.scalar.activation(
                    expS_w[:, 2 * jj : 2 * jj + 2, :], ps2, Exp, bias=0.0, scale=1.0
                )

            # ---- out2 -> e2 ----
            ps_o2 = pp_w.tile([P, CHW], F32, tag="w")
            for j in range(CHW):
                nc.tensor.matmul(
                    ps_o2[:, j : j + 1],
                    lhsT=c_b_w[:, j * P : (j + 1) * P],
                    rhs=w2b,
                    start=True,
                    stop=True,
                )
            e2col_w = win.tile([P, CHW], F32, tag="e2")
            nc.scalar.activation(e2col_w, ps_o2, Exp, bias=0.0, scale=1.0)

            # previous window's rowinv broadcast chain (recip(t-1) has long
            # finished; mid-window so it never heads the PE queue)
            if t > 0:
                rowinv_tail(t - 1)

            # ---- hoist next window's A/cast (ahead of this window's DVE
            # reduce/norm backlog) ----
            if t + 1 < T:
                hoist_prep(t + 1)

            # ---- cE = c^T * e2 ----
            cE_w = win.tile([P, CHW, P], BF16, tag="cE")
            for x in range(CHW // 2):
                ps_ct = pp_w.tile([P, 2, P], BF16, tag="w")
                for k in range(2):
                    j = 2 * x + k
                    nc.tensor.transpose(
                        ps_ct[:, k, :], c_b_w[:, j * P : (j + 1) * P], identity_b
                    )
                for k in range(2):
                    j = 2 * x + k
                    nc.vector.tensor_scalar(
                        cE_w[:, j, :],
                        ps_ct[:, k, :],
                        e2col_w[:, j : j + 1],
                        None,
                        mult,
                    )
            stash[t]["cE"] = cE_w
            stash[t]["expS"] = expS_w

            # ---- previous window's tmp accumulation ----
            if t > 0:
                tmp_mms(t - 1)
            if w == 0 and b == 1:
                emit_bprep(0)

            # ---- pass 1 ----
            for i in range(MCH):
                ps1 = pp_s.tile([P, WIN], F32, tag="s")
                for h in range(2):
                    nc.tensor.matmul(
                        ps1[:, h * M : (h + 1) * M],
                        lhsT=st["Bq"][:, i * P : (i + 1) * P],
                        rhs=c_b_w[:, h * M : (h + 1) * M],
                        start=True,
                        stop=True,
                    )
                nc.scalar.activation(
                    st["expST"][:, i, lo : lo + WIN],
                    ps1,
                    Exp,
                    bias=st["o1col"][:, i : i + 1],
                    scale=1.0,
                    accum_out=st["colsumU"][:, i, w : w + 1],
                )

            # ---- waves ----
            if b == 1:
                bwave_mm(0, w)
                bwave_finish()
            if t > 0:
                awave_mm(t - 1)
                awave_finish()

            # ---- rowsum -> rowinv columns (tail of the DVE queue; its
            # broadcast happens in the next window's prologue) ----
            rowsumC_w = win.tile([P, CHW], F32, tag="rs")
            nc.vector.reduce_sum(
                rowsumC_w[:, 0 : CHW // 2], expS_w[:, 0 : CHW // 2, :], axis=AxX
            )
            nc.vector.reduce_sum(
                rowsumC_w[:, CHW // 2 : CHW], expS_w[:, CHW // 2 : CHW, :], axis=AxX
            )
            rowprod_w = win.tile([P, CHW], F32, tag="rp")
            nc.vector.tensor_tensor(rowprod_w, rowsumC_w, e2col_w, mult)
            rowinvC_w = win.tile([P, CHW], F32, tag="ri")
            nc.vector.reciprocal(rowinvC_w, rowprod_w)
            stash[t]["riC"] = rowinvC_w

        # ---- fill ----
        prep_load(0)
        load_cwin(0)
        load_cwin(1)
        hoist_prep(0)
        for t in range(T):
            emit_window(t)
        # ---- drain ----
        rowinv_tail(T - 1)
        tmp_mms(T - 1)
        awave_mm(T - 1)
        awave_finish()
        emit_bprep(1)
        for w in range(NWIN):
            bwave_mm(1, w)
            bwave_finish()


_PROGRAM = None


def _build_program(loops=None):
    """Build the per-core Bass program. loops=None -> straight-line (grading
    path); loops=R -> wrap the body in a Tile For_i repetition loop (used
    only for steady-state benchmarking)."""
    nc = bacc.Bacc("TRN2", target_bir_lowering=False, debug=False, num_devices=NCORES)
    q_d = nc.dram_tensor("q", [CB, D, M], F32, kind="ExternalInput")
    c_d = nc.dram_tensor("c", [CB, D, N], F32, kind="ExternalInput")
    w1_d = nc.dram_tensor("w1", [1, D], F32, kind="ExternalInput")
    w2_d = nc.dram_tensor("w2", [1, D], F32, kind="ExternalInput")
    w3_d = nc.dram_tensor("w3", [1, D], F32, kind="ExternalInput")
    out_d = nc.dram_tensor("out", [CB, 4 * D, N], F32, kind="ExternalOutput")
    with tile.TileContext(nc) as tc:
        if loops is None:
            build_body(
                tc, q_d.ap(), c_d.ap(), w1_d.ap(), w2_d.ap(), w3_d.ap(), out_d.ap()
            )
        else:
            with tc.For_i(0, loops, 1):
                build_body(
                    tc,
                    q_d.ap(),
                    c_d.ap(),
                    w1_d.ap(),
                    w2_d.ap(),
                    w3_d.ap(),
                    out_d.ap(),
                )
    nc.compile()
    return nc


def _get_program():
    global _PROGRAM
    if _PROGRAM is None:
        _PROGRAM = _build_program()
    return _PROGRAM


def kernel(q, c, w1, w2, w3, _collect_results=None):
    q = np.ascontiguousarray(q, dtype=np.float32)
    c = np.ascontiguousarray(c, dtype=np.float32)
    w1 = np.ascontiguousarray(w1, dtype=np.float32)
    w2 = np.ascontiguousarray(w2, dtype=np.float32)
    w3 = np.ascontiguousarray(w3, dtype=np.float32)

    nc = _get_program()
    in_maps = [
        {
            "q": q[CB * i : CB * (i + 1)],
            "c": c[CB * i : CB * (i + 1)],
            "w1": w1,
            "w2": w2,
            "w3": w3,
        }
        for i in range(NCORES)
    ]
    from concourse import bass_utils

    res = bass_utils.run_bass_kernel_spmd(nc, in_maps, core_ids=list(range(NCORES)))
    if _collect_results is not None:
        _collect_results.append(res)
    return np.concatenate([r["out"] for r in res.results], axis=0)
